# revision 12
# baseline (speedup 1.0000x reference)
"""Trainium2 Bass kernel for AttentiveTransformer:
   out = sparsemax(GBN(feat @ W.T) * priors)

Data-parallel over 8 NeuronCores: batch 131072 rows -> 8 shards of 16384.
Per core: 128 chunks of VBS=128 rows, processed rows-on-partitions.

Per chunk c (128 rows):
  - PE transpose feat -> -featT (fp16, negated for free centering)
  - DVE reduce -> per-(i,chunk) feat sums
  - Scalar ACT: ftTc = 128*(featT - mean)  [scale=-128, bias=fsum]
  - PE gemm: xs = ftTc.T @ WTh -> PSUM f32 = 128*centered_x
  - Scalar ACT Square(xs/128) -> sq = xc^2 (fp16)
  - PE colsum: ones.T @ sq -> ssq[1,512] (variance*128)
  - Scalar ACT Rsqrt(128*ssq + 128^2*eps) -> s_row = isd/128 (fp16)
  - PE outer: ones_row.T @ s_row -> s_bc[128,512] PSUM (scale broadcast)
  - DVE: u = xs * s_bc = normalized x (fp16)
  - GpSimd: z = u * priors (fp16)
  - DVE top-16 via 4-segment MAX8 cascade + match_replace suppress
  - closed-form tau from cumsum scan; Scalar ACT relu(z - tau) -> out
"""
import sys

sys.path.insert(0, "/opt/trn_rl_repo")

import numpy as np
from contextlib import ExitStack

import concourse.bass as bass
import concourse.bacc as bacc
import concourse.tile as tile
from concourse import mybir
from concourse.bass_utils import run_bass_kernel_spmd

f32 = mybir.dt.float32
fp16 = mybir.dt.float16
AF = mybir.ActivationFunctionType
OP = mybir.AluOpType

N_CORES = 8
B, IN, D = 131072, 128, 512
ROWS = B // N_CORES          # 16384 rows per core
VBS = 128
N_CH = ROWS // VBS           # 128 chunks per core
N_SC = N_CH // 4             # 32 superchunks (DMA granularity)
EPS = 1e-5
NEG = -1.0e9


def build_nc(gamma_ones: bool, beta_zero: bool):
    nc = bacc.Bacc(None, target_bir_lowering=False)

    priors = nc.dram_tensor("priors", [ROWS, D], f32, kind="ExternalInput")
    feat = nc.dram_tensor("processed_feat", [ROWS, IN], f32, kind="ExternalInput")
    Wd = nc.dram_tensor("W", [D, IN], f32, kind="ExternalInput")
    gam = nc.dram_tensor("gamma", [D], f32, kind="ExternalInput")
    bet = nc.dram_tensor("beta", [D], f32, kind="ExternalInput")
    out = nc.dram_tensor("out", [ROWS, D], f32, kind="ExternalOutput")

    with tile.TileContext(nc) as tc, ExitStack() as ctx:
        singles = ctx.enter_context(tc.tile_pool(name="singles", bufs=1))
        fb_pool = ctx.enter_context(tc.tile_pool(name="fb", bufs=2))
        ftc_pool = ctx.enter_context(tc.tile_pool(name="ftc", bufs=8))
        p_pool = ctx.enter_context(tc.tile_pool(name="p", bufs=3))
        sq_pool = ctx.enter_context(tc.tile_pool(name="sq", bufs=4))
        sr_pool = ctx.enter_context(tc.tile_pool(name="sr", bufs=4))
        u_pool = ctx.enter_context(tc.tile_pool(name="u", bufs=4))
        z_pool = ctx.enter_context(tc.tile_pool(name="z", bufs=2))
        sm_pool = ctx.enter_context(tc.tile_pool(name="sm", bufs=4))
        o_pool = ctx.enter_context(tc.tile_pool(name="o", bufs=2))
        ps_ft = ctx.enter_context(tc.tile_pool(name="psft", bufs=1, space="PSUM"))
        ps_x = ctx.enter_context(tc.tile_pool(name="psx", bufs=2, space="PSUM"))
        ps_q = ctx.enter_context(tc.tile_pool(name="psq", bufs=2, space="PSUM"))
        ps_s = ctx.enter_context(tc.tile_pool(name="pss", bufs=3, space="PSUM"))

        # ---------- one-time constants ----------
        identh = singles.tile([128, 128], fp16)
        nc.gpsimd.iota(identh, [[1, 128]], base=0, channel_multiplier=-1,
                       allow_small_or_imprecise_dtypes=True)
        nc.vector.tensor_scalar(identh, identh, 0.0, None, OP.is_equal)

        ones_col = singles.tile([128, 1], fp16)
        nc.vector.memset(ones_col, 1.0)
        ones_row = singles.tile([1, 128], fp16)
        nc.vector.memset(ones_row, 1.0)

        # WTh [128 i, 512 d] fp16 resident
        WTh = singles.tile([128, D], fp16)
        wtp = ps_ft.tile([128, 4, 128], fp16, tag="ftp")
        for s in range(4):
            wtile = fb_pool.tile([128, 128], f32, tag="wtile")
            nc.sync.dma_start(out=wtile, in_=Wd[s * 128:(s + 1) * 128, :])
            wh = fb_pool.tile([128, 128], fp16, tag="wh")
            nc.vector.tensor_copy(wh, wtile)
            nc.tensor.transpose(wtp[:, s], wh, identh)
        nc.scalar.copy(WTh, wtp)

        # gamma/beta rows (general path only)
        if not gamma_ones:
            gamma_f = singles.tile([1, D], f32)
            nc.sync.dma_start(out=gamma_f,
                              in_=gam.rearrange("(o d) -> o d", o=1))
            gamma_row = singles.tile([1, D], fp16)
            nc.vector.tensor_copy(gamma_row, gamma_f)
        if not beta_zero:
            beta_f = singles.tile([1, D], f32)
            nc.sync.dma_start(out=beta_f,
                              in_=bet.rearrange("(o d) -> o d", o=1))
            beta_row = singles.tile([1, D], fp16)
            nc.vector.tensor_copy(beta_row, beta_f)
            beta_bc = singles.tile([128, D], fp16)
            nc.gpsimd.partition_broadcast(beta_bc, beta_row)

        # rho / invrho for tau formula, batched over 4 chunks
        rho16 = singles.tile([128, 16], f32)
        nc.gpsimd.iota(rho16, [[1, 16]], base=1, channel_multiplier=0,
                       allow_small_or_imprecise_dtypes=True)
        invrho = singles.tile([128, 16], f32)
        nc.vector.reciprocal(invrho, rho16)
        rho4 = singles.tile([128, 4, 16], f32)
        invrho4 = singles.tile([128, 4, 16], f32)
        for q in range(4):
            nc.vector.tensor_copy(rho4[:, q], rho16)
            nc.vector.tensor_copy(invrho4[:, q], invrho)

        fe_r = feat.rearrange("(n c p) k -> n p c k", p=128, c=4)
        pr_r = priors.rearrange("(n c p) d -> n p c d", p=128, c=4)
        out_r = out.rearrange("(n c p) d -> n p c d", p=128, c=4)

        # isd ACT: s_row = rsqrt(ssq/128 + eps); xsh holds true-scale xc
        ISD_SCALE = 1.0 / 128.0
        isd_bias = singles.tile([1, 1], f32)
        nc.vector.memset(isd_bias, EPS)

        st = {}

        def emit_sc_load(sc):
            f4 = fb_pool.tile([128, 4, 128], f32, tag="f4")
            nc.sync.dma_start(out=f4, in_=fe_r[sc])
            p4 = p_pool.tile([128, 4, D], f32, tag="p4")
            nc.sync.dma_start(out=p4, in_=pr_r[sc])
            fh = fb_pool.tile([128, 4, 128], fp16, tag="fh")
            # negate during fp16 convert: ftT holds -featT
            nc.vector.tensor_scalar(fh, f4, -1.0, None, OP.mult)
            ftp = ps_ft.tile([128, 4, 128], fp16, tag="ftp")
            for q in range(4):
                nc.tensor.transpose(ftp[:, q], fh[:, q], identh)
            fsum4 = sm_pool.tile([128, 4], f32, tag="fsum4")
            nc.vector.tensor_reduce(out=fsum4, in_=ftp,
                                    axis=mybir.AxisListType.X, op=OP.add)
            ftcs = []
            for q in range(4):
                # ftTc = (-featT)*(-128) + (-sum) = 128*(featT - mean)
                ftc = ftc_pool.tile([128, 128], fp16, tag="ftc")
                nc.scalar.activation(out=ftc, in_=ftp[:, q], func=AF.Identity,
                                     bias=fsum4[:, q:q + 1], scale=-128.0)
                ftcs.append(ftc)
            z4 = z_pool.tile([128, 4, D], f32, tag="z4")
            t16 = sm_pool.tile([128, 4, 16], f32, tag="t16")
            st[sc] = dict(p4=p4, ftcs=ftcs, z4=z4, t16=t16,
                          xsh={}, sq={}, srow={}, sbc={})

        def emit_gemm(c):
            sc, q = c // 4, c % 4
            s = st[sc]
            xs = ps_x.tile([128, D], f32, tag="xs")
            nc.tensor.matmul(xs, s["ftcs"][q], WTh)
            xsh = sq_pool.tile([128, D], f32, tag="xsh")
            nc.scalar.activation(out=xsh, in_=xs, func=AF.Identity,
                                 bias=0.0, scale=1.0 / 128.0)
            sq = sq_pool.tile([128, D], fp16, tag="sq")
            nc.gpsimd.tensor_tensor(sq, xsh, xsh, OP.mult)
            s["xsh"][q] = xsh
            s["sq"][q] = sq

        def emit_ssq(c):
            sc, q = c // 4, c % 4
            s = st[sc]
            ssq = ps_q.tile([1, D], f32, tag="ssq")
            nc.tensor.matmul(ssq, ones_col, s["sq"][q])
            srow = sr_pool.tile([1, D], fp16, tag="srow")
            nc.scalar.activation(out=srow, in_=ssq, func=AF.Abs_reciprocal_sqrt,
                                 bias=isd_bias, scale=ISD_SCALE)
            if not gamma_ones:
                nc.vector.tensor_tensor(srow, srow, gamma_row, OP.mult)
            s["srow"][q] = srow

        def emit_sbc(c):
            sc, q = c // 4, c % 4
            s = st[sc]
            sbc = ps_s.tile([128, D], f32, tag="sbc")
            nc.tensor.matmul(sbc, ones_row, s["srow"][q])
            s["sbc"][q] = sbc

        def emit_val(c):
            sc, q = c // 4, c % 4
            s = st[sc]
            # pp = priors * isd broadcast; z = xc * pp = xn * priors
            pp = u_pool.tile([128, D], f32, tag="pp")
            nc.vector.tensor_tensor(pp, s["p4"][:, q], s["sbc"][q], OP.mult)
            z = s["z4"][:, q]
            nc.gpsimd.tensor_tensor(z, s["xsh"][q], pp, OP.mult)
            del s["xsh"][q], s["sbc"][q], s["sq"][q]
            if not beta_zero:
                # z += beta * priors (general path; gpsimd is idle)
                bp = u_pool.tile([128, D], f32, tag="bp")
                nc.gpsimd.tensor_tensor(bp, beta_bc, s["p4"][:, q], OP.mult)
                nc.gpsimd.tensor_tensor(z, z, bp, OP.add)
            # top-16 cascade: 4 segment top-8s -> top8 -> suppress -> next8
            l1 = sm_pool.tile([128, 32], f32, tag="l1")
            for g in range(4):
                nc.vector.max(out=l1[:, g * 8:(g + 1) * 8],
                              in_=z[:, g * 128:(g + 1) * 128])
            t16 = s["t16"]
            nc.vector.max(out=t16[:, q, 0:8], in_=l1)
            sup = sm_pool.tile([128, 32], f32, tag="sup")
            nc.vector.match_replace(out=sup, in_to_replace=t16[:, q, 0:8],
                                    in_values=l1, imm_value=NEG)
            nc.vector.max(out=t16[:, q, 8:16], in_=sup)

        def emit_tail(sc):
            s = st[sc]
            t16 = s["t16"]
            cs4 = sm_pool.tile([128, 4, 16], f32, tag="cs4")
            for q in range(4):
                nc.vector.tensor_tensor_scan(cs4[:, q], t16[:, q], t16[:, q],
                                             -1.0, OP.add, OP.bypass)
            rz = sm_pool.tile([128, 4, 16], f32, tag="rz")
            nc.vector.tensor_tensor(rz, t16, rho4, OP.mult)
            cond = sm_pool.tile([128, 4, 17], f32, tag="cond")
            nc.gpsimd.memset(cond[:, :, 16:17], 0.0)
            nc.vector.tensor_tensor(cond[:, :, 0:16], rz, cs4, OP.is_gt)
            dcn = sm_pool.tile([128, 4, 16], f32, tag="dcn")
            nc.vector.tensor_tensor(dcn, cond[:, :, 1:17], cond[:, :, 0:16],
                                    OP.subtract)
            scr = sm_pool.tile([128, 4, 16], f32, tag="scr")
            nc.vector.tensor_tensor(scr, cs4, invrho4, OP.mult)
            nc.vector.tensor_tensor(scr, scr, dcn, OP.mult)
            negtau = sm_pool.tile([128, 4], f32, tag="negtau")
            nc.vector.tensor_reduce(out=negtau, in_=scr,
                                    axis=mybir.AxisListType.X, op=OP.add)
            o4 = o_pool.tile([128, 4, D], f32, tag="o4")
            for q in range(4):
                nc.scalar.activation(out=o4[:, q], in_=s["z4"][:, q],
                                     func=AF.Relu, bias=negtau[:, q:q + 1],
                                     scale=1.0)
            nc.sync.dma_start(out=out_r[sc], in_=o4)
            del st[sc]

        # ---------- stage-skewed main loop ----------
        for c in range(N_CH + 4):
            if c % 4 == 0 and c // 4 < N_SC:
                emit_sc_load(c // 4)
            if c < N_CH:
                emit_gemm(c)
            if 2 <= c and c - 2 < N_CH:
                emit_ssq(c - 2)
            if 3 <= c and c - 3 < N_CH:
                emit_sbc(c - 3)
            if 4 <= c and c - 4 < N_CH:
                emit_val(c - 4)
                if (c - 4) % 4 == 3:
                    emit_tail((c - 4) // 4)

    nc.compile()
    return nc


_NC_CACHE = {}


def kernel(**inputs) -> np.ndarray:
    priors = np.ascontiguousarray(inputs["priors"], dtype=np.float32)
    feat = np.ascontiguousarray(inputs["processed_feat"], dtype=np.float32)
    W = np.ascontiguousarray(inputs["W"], dtype=np.float32)
    gamma = np.ascontiguousarray(inputs["gamma"], dtype=np.float32)
    beta = np.ascontiguousarray(inputs["beta"], dtype=np.float32)

    key = (bool(np.all(gamma == 1.0)), bool(np.all(beta == 0.0)))
    if key not in _NC_CACHE:
        _NC_CACHE[key] = build_nc(*key)
    nc = _NC_CACHE[key]

    in_maps = []
    for i in range(N_CORES):
        sl = slice(i * ROWS, (i + 1) * ROWS)
        in_maps.append({
            "priors": priors[sl],
            "processed_feat": feat[sl],
            "W": W,
            "gamma": gamma,
            "beta": beta,
        })
    res = run_bass_kernel_spmd(nc, in_maps, core_ids=list(range(N_CORES)))
    return np.concatenate([r["out"] for r in res.results], axis=0)


if __name__ == "__main__":
    rng = np.random.default_rng(0)
    inputs = {
        "priors": rng.random((B, D), dtype=np.float32),
        "processed_feat": rng.standard_normal((B, IN), dtype=np.float32),
        "W": (rng.standard_normal((D, IN), dtype=np.float32) * 0.1),
        "gamma": np.ones(D, dtype=np.float32),
        "beta": np.zeros(D, dtype=np.float32),
    }
    out = kernel(**inputs)
    print("out", out.shape, out.dtype, float(out.sum()))


# revision 14
# speedup vs baseline: 1.0352x; 1.0352x over previous
"""Trainium2 Bass kernel for AttentiveTransformer:
   out = sparsemax(GBN(feat @ W.T) * priors)

Data-parallel over 8 NeuronCores: batch 131072 rows -> 8 shards of 16384.
Per core: 128 chunks of VBS=128 rows, processed rows-on-partitions.

Per chunk c (128 rows):
  - PE transpose feat -> -featT (fp16, negated for free centering)
  - DVE reduce -> per-(i,chunk) feat sums
  - Scalar ACT: ftTc = 128*(featT - mean)  [scale=-128, bias=fsum]
  - PE gemm: xs = ftTc.T @ WTh -> PSUM f32 = 128*centered_x
  - Scalar ACT Square(xs/128) -> sq = xc^2 (fp16)
  - PE colsum: ones.T @ sq -> ssq[1,512] (variance*128)
  - Scalar ACT Rsqrt(128*ssq + 128^2*eps) -> s_row = isd/128 (fp16)
  - PE outer: ones_row.T @ s_row -> s_bc[128,512] PSUM (scale broadcast)
  - DVE: u = xs * s_bc = normalized x (fp16)
  - GpSimd: z = u * priors (fp16)
  - DVE top-16 via 4-segment MAX8 cascade + match_replace suppress
  - closed-form tau from cumsum scan; Scalar ACT relu(z - tau) -> out
"""
import sys

sys.path.insert(0, "/opt/trn_rl_repo")

import numpy as np
from contextlib import ExitStack

import concourse.bass as bass
import concourse.bacc as bacc
import concourse.tile as tile
from concourse import mybir
from concourse.bass_utils import run_bass_kernel_spmd

f32 = mybir.dt.float32
fp16 = mybir.dt.float16
AF = mybir.ActivationFunctionType
OP = mybir.AluOpType

N_CORES = 8
B, IN, D = 131072, 128, 512
ROWS = B // N_CORES          # 16384 rows per core
VBS = 128
N_CH = ROWS // VBS           # 128 chunks per core
N_SC = N_CH // 4             # 32 superchunks (DMA granularity)
EPS = 1e-5
NEG = -1.0e9


def build_nc(gamma_ones: bool, beta_zero: bool):
    nc = bacc.Bacc(None, target_bir_lowering=False)

    priors = nc.dram_tensor("priors", [ROWS, D], f32, kind="ExternalInput")
    feat = nc.dram_tensor("processed_feat", [ROWS, IN], f32, kind="ExternalInput")
    Wd = nc.dram_tensor("W", [D, IN], f32, kind="ExternalInput")
    gam = nc.dram_tensor("gamma", [D], f32, kind="ExternalInput")
    bet = nc.dram_tensor("beta", [D], f32, kind="ExternalInput")
    out = nc.dram_tensor("out", [ROWS, D], f32, kind="ExternalOutput")

    with tile.TileContext(nc) as tc, ExitStack() as ctx:
        singles = ctx.enter_context(tc.tile_pool(name="singles", bufs=1))
        fb_pool = ctx.enter_context(tc.tile_pool(name="fb", bufs=2))
        ftc_pool = ctx.enter_context(tc.tile_pool(name="ftc", bufs=8))
        p_pool = ctx.enter_context(tc.tile_pool(name="p", bufs=3))
        sq_pool = ctx.enter_context(tc.tile_pool(name="sq", bufs=4))
        sr_pool = ctx.enter_context(tc.tile_pool(name="sr", bufs=4))
        u_pool = ctx.enter_context(tc.tile_pool(name="u", bufs=4))
        z_pool = ctx.enter_context(tc.tile_pool(name="z", bufs=2))
        sm_pool = ctx.enter_context(tc.tile_pool(name="sm", bufs=4))
        o_pool = ctx.enter_context(tc.tile_pool(name="o", bufs=2))
        ps_ft = ctx.enter_context(tc.tile_pool(name="psft", bufs=1, space="PSUM"))
        ps_x = ctx.enter_context(tc.tile_pool(name="psx", bufs=3, space="PSUM"))
        ps_q = ctx.enter_context(tc.tile_pool(name="psq", bufs=2, space="PSUM"))
        ps_s = ctx.enter_context(tc.tile_pool(name="pss", bufs=2, space="PSUM"))

        # ---------- one-time constants ----------
        identh = singles.tile([128, 128], fp16)
        nc.gpsimd.iota(identh, [[1, 128]], base=0, channel_multiplier=-1,
                       allow_small_or_imprecise_dtypes=True)
        nc.vector.tensor_scalar(identh, identh, 0.0, None, OP.is_equal)

        ones_col = singles.tile([128, 1], fp16)
        nc.vector.memset(ones_col, 1.0)
        ones_row = singles.tile([1, 128], fp16)
        nc.vector.memset(ones_row, 1.0)

        # WTh [128 i, 512 d] fp16 resident
        WTh = singles.tile([128, D], fp16)
        wtp = ps_ft.tile([128, 4, 128], fp16, tag="ftp")
        for s in range(4):
            wtile = fb_pool.tile([128, 128], f32, tag="wtile")
            nc.sync.dma_start(out=wtile, in_=Wd[s * 128:(s + 1) * 128, :])
            wh = fb_pool.tile([128, 128], fp16, tag="wh")
            nc.vector.tensor_copy(wh, wtile)
            nc.tensor.transpose(wtp[:, s], wh, identh)
        nc.scalar.copy(WTh, wtp)

        # gamma/beta rows (general path only)
        if not gamma_ones:
            gamma_f = singles.tile([1, D], f32)
            nc.sync.dma_start(out=gamma_f,
                              in_=gam.rearrange("(o d) -> o d", o=1))
            gamma_row = singles.tile([1, D], fp16)
            nc.vector.tensor_copy(gamma_row, gamma_f)
        if not beta_zero:
            beta_f = singles.tile([1, D], f32)
            nc.sync.dma_start(out=beta_f,
                              in_=bet.rearrange("(o d) -> o d", o=1))
            beta_row = singles.tile([1, D], fp16)
            nc.vector.tensor_copy(beta_row, beta_f)
            beta_bc = singles.tile([128, D], fp16)
            nc.gpsimd.partition_broadcast(beta_bc, beta_row)

        # rho / invrho for tau formula, batched over 4 chunks
        rho16 = singles.tile([128, 16], f32)
        nc.gpsimd.iota(rho16, [[1, 16]], base=1, channel_multiplier=0,
                       allow_small_or_imprecise_dtypes=True)
        ninvrho = singles.tile([128, 16], f32)
        nc.vector.reciprocal(ninvrho, rho16)
        nc.vector.tensor_scalar(ninvrho, ninvrho, -1.0, None, OP.mult)
        ninvrho4 = singles.tile([128, 4, 16], f32)
        for q in range(4):
            nc.vector.tensor_copy(ninvrho4[:, q], ninvrho)

        fe_r = feat.rearrange("(n c p) k -> n p c k", p=128, c=4)
        pr_r = priors.rearrange("(n c p) d -> n p c d", p=128, c=4)
        out_r = out.rearrange("(n c p) d -> n p c d", p=128, c=4)

        # isd ACT: s_row = (1/128)*rsqrt(ssq/128 + eps) = 1/sqrt(128*ssq+16384*eps)
        # (the 1/128 compensates xs = 128*xc)
        ISD_SCALE = 128.0
        isd_bias = singles.tile([1, 1], f32)
        nc.vector.memset(isd_bias, 16384.0 * EPS)

        st = {}

        def emit_sc_load(sc):
            f4 = fb_pool.tile([128, 4, 128], f32, tag="f4")
            nc.sync.dma_start(out=f4, in_=fe_r[sc])
            p4 = p_pool.tile([128, 4, D], f32, tag="p4")
            nc.sync.dma_start(out=p4, in_=pr_r[sc])
            fh = fb_pool.tile([128, 4, 128], fp16, tag="fh")
            # negate during fp16 convert: ftT holds -featT
            nc.vector.tensor_scalar(fh, f4, -1.0, None, OP.mult)
            ftp = ps_ft.tile([128, 4, 128], fp16, tag="ftp")
            for q in range(4):
                nc.tensor.transpose(ftp[:, q], fh[:, q], identh)
            fsum4 = sm_pool.tile([128, 4], f32, tag="fsum4")
            nc.vector.tensor_reduce(out=fsum4, in_=ftp,
                                    axis=mybir.AxisListType.X, op=OP.add)
            ftcs = []
            for q in range(4):
                # ftTc = (-featT)*(-128) + (-sum) = 128*(featT - mean)
                ftc = ftc_pool.tile([128, 128], fp16, tag="ftc")
                nc.scalar.activation(out=ftc, in_=ftp[:, q], func=AF.Identity,
                                     bias=fsum4[:, q:q + 1], scale=-128.0)
                ftcs.append(ftc)
            z4 = z_pool.tile([128, 4, D], f32, tag="z4")
            t16 = sm_pool.tile([128, 4, 16], f32, tag="t16")
            st[sc] = dict(p4=p4, ftcs=ftcs, z4=z4, t16=t16,
                          xs={}, sq={}, srow={}, sbc={})

        def emit_gemm(c):
            sc, q = c // 4, c % 4
            s = st[sc]
            xs = ps_x.tile([128, D], f32, tag="xs")
            nc.tensor.matmul(xs, s["ftcs"][q], WTh)
            sq = sq_pool.tile([128, D], fp16, tag="sq")
            nc.scalar.activation(out=sq, in_=xs, func=AF.Square,
                                 bias=0.0, scale=1.0 / 128.0)
            s["xs"][q] = xs
            s["sq"][q] = sq

        def emit_ssq(c):
            sc, q = c // 4, c % 4
            s = st[sc]
            ssq = ps_q.tile([1, D], f32, tag="ssq")
            nc.tensor.matmul(ssq, ones_col, s["sq"][q])
            srow = sr_pool.tile([1, D], fp16, tag="srow")
            nc.scalar.activation(out=srow, in_=ssq, func=AF.Abs_reciprocal_sqrt,
                                 bias=isd_bias, scale=ISD_SCALE)
            if not gamma_ones:
                nc.vector.tensor_tensor(srow, srow, gamma_row, OP.mult)
            s["srow"][q] = srow

        def emit_sbc(c):
            sc, q = c // 4, c % 4
            s = st[sc]
            sbc = ps_s.tile([128, D], f32, tag="sbc")
            nc.tensor.matmul(sbc, ones_row, s["srow"][q])
            s["sbc"][q] = sbc

        def emit_val(c):
            sc, q = c // 4, c % 4
            s = st[sc]
            # pp = priors * isd broadcast; z = 128*xc * pp = xn * priors
            pp = u_pool.tile([128, D], f32, tag="pp")
            nc.vector.tensor_tensor(pp, s["p4"][:, q], s["sbc"][q], OP.mult)
            z = s["z4"][:, q]
            nc.vector.tensor_tensor(z, s["xs"][q], pp, OP.mult)
            del s["xs"][q], s["sbc"][q], s["sq"][q]
            if not beta_zero:
                # z += beta * priors (general path; gpsimd is idle)
                bp = u_pool.tile([128, D], f32, tag="bp")
                nc.gpsimd.tensor_tensor(bp, beta_bc, s["p4"][:, q], OP.mult)
                nc.gpsimd.tensor_tensor(z, z, bp, OP.add)
            # top-16 cascade: 4 segment top-8s -> top8 -> suppress -> next8
            l1 = sm_pool.tile([128, 32], f32, tag="l1")
            for g in range(4):
                nc.vector.max(out=l1[:, g * 8:(g + 1) * 8],
                              in_=z[:, g * 128:(g + 1) * 128])
            t16 = s["t16"]
            nc.vector.max(out=t16[:, q, 0:8], in_=l1)
            sup = sm_pool.tile([128, 32], f32, tag="sup")
            nc.vector.match_replace(out=sup, in_to_replace=t16[:, q, 0:8],
                                    in_values=l1, imm_value=NEG)
            nc.vector.max(out=t16[:, q, 8:16], in_=sup)

        def emit_tail(sc):
            s = st[sc]
            t16 = s["t16"]
            cs4 = sm_pool.tile([128, 4, 16], f32, tag="cs4")
            for q in range(4):
                nc.vector.tensor_tensor_scan(cs4[:, q], t16[:, q], t16[:, q],
                                             -1.0, OP.add, OP.bypass)
            # sparsemax identity: tau = max_j (cumsum_j - 1)/j
            scr = sm_pool.tile([128, 4, 16], f32, tag="scr")
            nc.vector.tensor_tensor(scr, cs4, ninvrho4, OP.mult)
            negtau = sm_pool.tile([128, 4], f32, tag="negtau")
            nc.vector.tensor_reduce(out=negtau, in_=scr,
                                    axis=mybir.AxisListType.X, op=OP.min)
            o4 = o_pool.tile([128, 4, D], f32, tag="o4")
            for q in range(4):
                nc.scalar.activation(out=o4[:, q], in_=s["z4"][:, q],
                                     func=AF.Relu, bias=negtau[:, q:q + 1],
                                     scale=1.0)
            nc.sync.dma_start(out=out_r[sc], in_=o4)
            del st[sc]

        # ---------- stage-skewed main loop ----------
        for c in range(N_CH + 3):
            if c % 4 == 0 and c // 4 < N_SC:
                emit_sc_load(c // 4)
            if c < N_CH:
                emit_gemm(c)
            if 1 <= c and c - 1 < N_CH:
                emit_ssq(c - 1)
            if 2 <= c and c - 2 < N_CH:
                emit_sbc(c - 2)
            if 3 <= c and c - 3 < N_CH:
                emit_val(c - 3)
                if (c - 3) % 4 == 3:
                    emit_tail((c - 3) // 4)

    nc.compile()
    return nc


_NC_CACHE = {}


def kernel(**inputs) -> np.ndarray:
    priors = np.ascontiguousarray(inputs["priors"], dtype=np.float32)
    feat = np.ascontiguousarray(inputs["processed_feat"], dtype=np.float32)
    W = np.ascontiguousarray(inputs["W"], dtype=np.float32)
    gamma = np.ascontiguousarray(inputs["gamma"], dtype=np.float32)
    beta = np.ascontiguousarray(inputs["beta"], dtype=np.float32)

    key = (bool(np.all(gamma == 1.0)), bool(np.all(beta == 0.0)))
    if key not in _NC_CACHE:
        _NC_CACHE[key] = build_nc(*key)
    nc = _NC_CACHE[key]

    in_maps = []
    for i in range(N_CORES):
        sl = slice(i * ROWS, (i + 1) * ROWS)
        in_maps.append({
            "priors": priors[sl],
            "processed_feat": feat[sl],
            "W": W,
            "gamma": gamma,
            "beta": beta,
        })
    res = run_bass_kernel_spmd(nc, in_maps, core_ids=list(range(N_CORES)))
    return np.concatenate([r["out"] for r in res.results], axis=0)


if __name__ == "__main__":
    rng = np.random.default_rng(0)
    inputs = {
        "priors": rng.random((B, D), dtype=np.float32),
        "processed_feat": rng.standard_normal((B, IN), dtype=np.float32),
        "W": (rng.standard_normal((D, IN), dtype=np.float32) * 0.1),
        "gamma": np.ones(D, dtype=np.float32),
        "beta": np.zeros(D, dtype=np.float32),
    }
    out = kernel(**inputs)
    print("out", out.shape, out.dtype, float(out.sum()))
